# revision 38
# baseline (speedup 1.0000x reference)
"""Trainium2 Bass kernel: analytical Hessian of the ARAP energy w.r.t. a latent code.

Math (derived from the reference, exact because relu'' == 0 a.e.):
    wt[p,j] = weightMatrix[p,j] * (j < numNeighbors[p])          [N, K]
    s       = (code @ W1 + b1 > 0)                               [H]
    X       = (W1 * s) @ W2   viewed [NZ, N*3]                   (the Jacobian d recon/d code)
    L       = D - S - S^T     (graph Laplacian; S[p, n[p,j]] += wt[p,j],
                               D = diag(rowsum(S) + colsum(S)))
    Hess    = (2/(N*K)) * X (L (x) I3) X^T                       [NZ, NZ]

Two structural identities shape the kernel:
  1. X (L (x) I3) = U @ (W2 (L (x) I3)): the sparse Laplacian application is a
     fixed recombination of W2's columns by the static, input-derived edge
     weights -- precomputed once on the host as W2L (the device's hardware
     gather paths are unusable in this stack; the matmul mass stays on device).
  2. U = W1 * s has zero columns wherever the relu is inactive -- those rows of
     W2 / W2L contribute nothing, so only the ~H/2 active rows are shipped and
     multiplied (structured sparsity; prep_inputs derives the mask from the
     actual runtime inputs, so this is exact for any inputs).

Per core (vertices column-sharded, 625/core; HP = padded active-row count):
    stage 1a:  XT_c = (W2 active block)^T  @ U_active    NCH chunks x HP/128 K-tiles
    stage 1b:  YT_c = (W2L active block)^T @ U_active    NCH chunks x HP/128 K-tiles
    stage 3 :  psH += XT_c (contract rows) YT_c          NCH accumulating matmuls
Per-core partial Hessians are summed on the host (times 2/(N*K)).
W2/W2L chunks stream via per-chunk DMAs so TensorE starts ~2us in.
"""

import numpy as np

import sys

for _p in ("/opt/trn_rl_repo", "/root/.axon_site/_ro/trn_rl_repo"):
    if _p not in sys.path:
        sys.path.insert(0, _p)

from concourse import bass, mybir
from concourse.bass_utils import run_bass_kernel_spmd

F16 = np.float16

N, K, NZ, H = 5000, 20, 128, 1024
NCORES = 8
VPC = N // NCORES            # 625 vertices per core
RLOC = VPC * 3               # 1875 live rows per core
NCH = 15                     # (p,a)-row chunks of 128 per core
RPAD = NCH * 128             # 1920 padded rows per core
SCALE = 2.0 / (N * K)


def build_graph(nt, na):
    """nt K-tiles over na active hidden units; last tile may be partial."""
    tk = [min(128, na - 128 * t) for t in range(nt)]
    nc = bass.Bass(target_bir_lowering=False)

    f32 = mybir.dt.float32
    f16 = mybir.dt.float16

    ut_p = nc.declare_dram_parameter("ut", [128, nt * 128], f16, isOutput=False)
    w2a = nc.declare_dram_parameter(
        "w2a", [128, NCH, 2, nt, 128], f16, isOutput=False
    )
    out_p = nc.declare_dram_parameter("out", [128, 128], f32, isOutput=True)

    from contextlib import ExitStack

    with ExitStack() as ctx:
        block = ctx.enter_context(nc.Block(no_gpsimd_drain=True))
        sem_ut = ctx.enter_context(nc.semaphore("sem_ut"))
        sem_x = ctx.enter_context(nc.semaphore("sem_x"))
        sem_xc = ctx.enter_context(nc.semaphore("sem_xc"))
        sem_h = ctx.enter_context(nc.semaphore("sem_h"))
        sem_fin = ctx.enter_context(nc.semaphore("sem_fin"))
        sem_outd = ctx.enter_context(nc.semaphore("sem_outd"))
        semw = [ctx.enter_context(nc.semaphore(f"semw{i}")) for i in range(NCH)]
        semw0x = ctx.enter_context(nc.semaphore("semw0x"))
        wthr = 16 * (2 if tk[-1] < 128 else 1)  # per-chunk DMA count
        # note: semw0x and semw[0] each count their own X/Y halves
        sb_ut = ctx.enter_context(nc.sbuf_tensor("sb_ut", [128, nt * 128], f16))
        sb_w2a = ctx.enter_context(
            nc.sbuf_tensor("sb_w2a", [128, NCH, 2, nt, 128], f16)
        )
        sb_xt = ctx.enter_context(nc.sbuf_tensor("sb_xt", [128, NCH * 128], f16))
        sb_yt = ctx.enter_context(nc.sbuf_tensor("sb_yt", [128, NCH * 128], f16))
        sb_out = ctx.enter_context(nc.sbuf_tensor("sb_out", [128, 128], f32))
        psXa = ctx.enter_context(nc.psum_tensor("psXa", [128, 128], f32))
        psXb = ctx.enter_context(nc.psum_tensor("psXb", [128, 128], f32))
        psYa = ctx.enter_context(nc.psum_tensor("psYa", [128, 128], f32))
        psYb = ctx.enter_context(nc.psum_tensor("psYb", [128, 128], f32))
        psH = ctx.enter_context(nc.psum_tensor("psH", [128, 128], f32))
        psW = ctx.enter_context(nc.psum_tensor("psW", [128, 128], f32))
        psX = [psXa, psXb]
        psY = [psYa, psYb]

        nfull = sum(1 for k in tk if k == 128)
        ptk = tk[-1] if tk[-1] < 128 else None

        def _chunk_dma(eng, ch):
            eng.dma_start(
                out=sb_w2a[:, ch, :, :nfull, :], in_=w2a[:, ch, :, :nfull, :]
            ).then_inc(semw[ch], 16)
            if ptk is not None:
                eng.dma_start(
                    out=sb_w2a[:ptk, ch, :, nfull, :],
                    in_=w2a[:ptk, ch, :, nfull, :],
                ).then_inc(semw[ch], 16)

        @block.scalar
        def _(scalar: bass.BassScalarEngine):
            # U_active first on the ACT HWDGE ring, then its chunk share
            scalar.dma_start(out=sb_ut[:, :], in_=ut_p[:, :]).then_inc(sem_ut, 16)
            for ch in range(NCH):
                if ch % 3 == 1:
                    _chunk_dma(scalar, ch)


        @block.sync
        def _(sync: bass.BassEngine):
            sync.dma_start(
                out=sb_w2a[:, 0, 0, :nfull, :], in_=w2a[:, 0, 0, :nfull, :]
            ).then_inc(semw0x, 16)
            if ptk is not None:
                sync.dma_start(
                    out=sb_w2a[:ptk, 0, 0, nfull, :],
                    in_=w2a[:ptk, 0, 0, nfull, :],
                ).then_inc(semw0x, 16)
            sync.dma_start(
                out=sb_w2a[:, 0, 1, :nfull, :], in_=w2a[:, 0, 1, :nfull, :]
            ).then_inc(semw[0], 16)
            if ptk is not None:
                sync.dma_start(
                    out=sb_w2a[:ptk, 0, 1, nfull, :],
                    in_=w2a[:ptk, 0, 1, nfull, :],
                ).then_inc(semw[0], 16)
            for ch in range(NCH):
                if ch % 3 == 0 and ch != 0:
                    _chunk_dma(sync, ch)
            sync.wait_ge(sem_fin, 1)
            sync.dma_start(out=out_p[:, :], in_=sb_out[:, :]).then_inc(sem_outd, 16)
            sync.wait_ge(sem_outd, 16)

        @block.gpsimd
        def _(gpsimd: bass.BassGpSimd):
            for ch in range(NCH):
                if ch % 3 == 2:
                    _chunk_dma(gpsimd, ch)

        @block.tensor
        def _(tensor: bass.BassTensorEngine):
            tensor.wait_ge(sem_ut, 16)
            # HAM warmup while chunk 0 is still in flight
            for w in range(24):
                tensor.matmul(
                    psW[:, :],
                    lhsT=sb_ut[:, 0:128],
                    rhs=sb_ut[:, 0:128],
                    start=True,
                    stop=True,
                )
            def _s3(ch):
                return tensor.matmul(
                    psH[:, :],
                    lhsT=sb_xt[:, ch * 128 : (ch + 1) * 128],
                    rhs=sb_yt[:, ch * 128 : (ch + 1) * 128],
                    start=(ch == 0),
                    stop=(ch == NCH - 1),
                )

            for ch in range(NCH):
                if ch >= 2:
                    tensor.wait_ge(sem_xc, 2 * (ch - 1))
                    _s3(ch - 2)  # stage-3 for ch-2 (its copies are done)
                tensor.wait_ge(semw0x if ch == 0 else semw[ch], wthr)
                for t in range(nt):
                    ins = tensor.matmul(
                        psX[ch % 2][:, :],
                        lhsT=sb_w2a[: tk[t], ch, 0, t, :],
                        rhs=sb_ut[: tk[t], t * 128 : (t + 1) * 128],
                        start=(t == 0),
                        stop=(t == nt - 1),
                    )
                ins.then_inc(sem_x, 1)
                if ch == 0:
                    tensor.wait_ge(semw[0], wthr)
                for t in range(nt):
                    ins = tensor.matmul(
                        psY[ch % 2][:, :],
                        lhsT=sb_w2a[: tk[t], ch, 1, t, :],
                        rhs=sb_ut[: tk[t], t * 128 : (t + 1) * 128],
                        start=(t == 0),
                        stop=(t == nt - 1),
                    )
                ins.then_inc(sem_x, 1)
            for ch in (NCH - 2, NCH - 1):
                tensor.wait_ge(sem_xc, 2 * (ch + 1))
                ins = _s3(ch)
            ins.then_inc(sem_h, 1)

        @block.vector
        def _(vector: bass.BassVectorEngine):
            # PSUM -> SBUF f16 copies of stage-1 chunks (X then Y per chunk)
            for ch in range(NCH):
                vector.wait_ge(sem_x, 2 * ch + 1)
                vector.tensor_copy(
                    sb_xt[:, ch * 128 : (ch + 1) * 128], psX[ch % 2][:, :]
                ).then_inc(sem_xc, 1)
                vector.wait_ge(sem_x, 2 * ch + 2)
                vector.tensor_copy(
                    sb_yt[:, ch * 128 : (ch + 1) * 128], psY[ch % 2][:, :]
                ).then_inc(sem_xc, 1)
            vector.wait_ge(sem_h, 1)
            vector.tensor_copy(sb_out[:, :], psH[:, :]).then_inc(sem_fin, 1)

    return nc


def prep_inputs(code, xyz1, weightMatrix, W1, b1, W2, b2, neighborsMatrix, numNeighbors):
    """Host-side sharding/layout prep. Returns (in_maps, nt)."""
    code = np.asarray(code, np.float64)
    W1 = np.asarray(W1, np.float64)
    W2 = np.asarray(W2, np.float32)
    b1 = np.asarray(b1, np.float64)
    wM = np.asarray(weightMatrix, np.float32)
    nbr = np.asarray(neighborsMatrix, np.int64)
    nn = np.asarray(numNeighbors, np.int64)

    mask = (np.arange(K)[None, :] < nn[:, None]).astype(np.float64)
    wt = np.asarray(wM, np.float64) * mask              # [N, K]

    # relu mask -> active hidden units (zero columns of U drop out exactly)
    z = (code @ W1 + b1)[0]
    act = np.where(z > 0)[0]
    na = len(act)
    nt = max(1, (na + 127) // 128)
    HP = nt * 128

    # W2L = W2 (L (x) I3)
    W2vT = np.ascontiguousarray(
        W2.astype(np.float32).reshape(H, N, 3).transpose(1, 2, 0)
    )                                                   # [N, 3, H]
    deg_out = wt.sum(1)
    deg_in = np.zeros(N)
    np.add.at(deg_in, nbr.ravel(), wt.ravel())
    d_tot = (deg_out + deg_in).astype(np.float32)

    W2LvT = W2vT * d_tot[:, None, None]
    wt32 = wt.astype(np.float32)
    for j in range(K):
        nj, wj = nbr[:, j], wt32[:, j]
        W2LvT -= wj[:, None, None] * W2vT[nj]                    # S term
        np.add.at(W2LvT, nj, -(wj[:, None, None] * W2vT))        # S^T term

    # active-row selection, padded to HP
    W2a = np.zeros((HP, N * 3), np.float32)
    W2a[:na] = W2.reshape(H, N * 3)[act]
    W2La = np.zeros((HP, N * 3), np.float32)
    W2La[:na] = W2LvT.transpose(2, 0, 1).reshape(H, N * 3)[act]

    # U_active^T tiles: ut[p, t*128+k] = W1[k, act[t*128+p]]  (pad rows zero)
    ut_h = np.zeros((HP, NZ), np.float32)
    ut_h[:na] = W1.T[act]
    ut_h = np.ascontiguousarray(
        ut_h.reshape(nt, 128, NZ).transpose(1, 0, 2).reshape(128, nt * NZ)
    ).astype(F16)

    def col_block(M, c):
        blk = np.zeros((HP, RPAD), np.float32)
        blk[:, :RLOC] = M[:, 3 * c * VPC : 3 * c * VPC + RLOC]
        # [part, ch, t, col] = blk[t*128+part, ch*128+col]
        return blk.reshape(nt, 128, NCH, 128).transpose(1, 2, 0, 3)

    in_maps = []
    for c in range(NCORES):
        both = np.stack([col_block(W2a, c), col_block(W2La, c)], axis=2)
        in_maps.append(
            {
                "ut": ut_h,
                "w2a": np.ascontiguousarray(both).astype(F16),
            }
        )
    return in_maps, nt, na


_CACHED = {}


def run_on_hw(in_maps, nt, na, trace=False):
    if (nt, na) not in _CACHED:
        _CACHED[(nt, na)] = build_graph(nt, na)
    res = run_bass_kernel_spmd(
        _CACHED[(nt, na)], in_maps, core_ids=list(range(NCORES)), trace=trace
    )
    return res


def assemble(parts):
    m = np.sum([np.asarray(p, np.float64) for p in parts], axis=0)
    return (m * SCALE).astype(np.float32)


def kernel(**inputs):
    in_maps, nt, na = prep_inputs(**inputs)
    res = run_on_hw(in_maps, nt, na)
    return assemble([res.results[c]["out"] for c in range(NCORES)])


if __name__ == "__main__":
    import reference

    inputs = {k: np.asarray(v) for k, v in reference.setup_inputs().items()}
    out = kernel(**inputs)
    print("out shape", out.shape, "absmax", np.abs(out).max())


# revision 39
# speedup vs baseline: 1.1024x; 1.1024x over previous
"""Trainium2 Bass kernel: analytical Hessian of the ARAP energy w.r.t. a latent code.

Math (derived from the reference, exact because relu'' == 0 a.e.):
    wt[p,j] = weightMatrix[p,j] * (j < numNeighbors[p])          [N, K]
    s       = (code @ W1 + b1 > 0)                               [H]
    X       = (W1 * s) @ W2   viewed [NZ, N*3]                   (the Jacobian d recon/d code)
    L       = D - S - S^T     (graph Laplacian; S[p, n[p,j]] += wt[p,j],
                               D = diag(rowsum(S) + colsum(S)))
    Hess    = (2/(N*K)) * X (L (x) I3) X^T                       [NZ, NZ]

Two structural identities shape the kernel:
  1. X (L (x) I3) = U @ (W2 (L (x) I3)): the sparse Laplacian application is a
     fixed recombination of W2's columns by the static, input-derived edge
     weights -- precomputed once on the host as W2L (the device's hardware
     gather paths are unusable in this stack; the matmul mass stays on device).
  2. U = W1 * s has zero columns wherever the relu is inactive -- those rows of
     W2 / W2L contribute nothing, so only the ~H/2 active rows are shipped and
     multiplied (structured sparsity; prep_inputs derives the mask from the
     actual runtime inputs, so this is exact for any inputs).

Per core (vertices column-sharded, 625/core; HP = padded active-row count):
    stage 1a:  XT_c = (W2 active block)^T  @ U_active    NCH chunks x HP/128 K-tiles
    stage 1b:  YT_c = (W2L active block)^T @ U_active    NCH chunks x HP/128 K-tiles
    stage 3 :  psH += XT_c (contract rows) YT_c          NCH accumulating matmuls
Per-core partial Hessians are summed on the host (times 2/(N*K)).
W2/W2L chunks stream via per-chunk DMAs so TensorE starts ~2us in.
"""

import numpy as np

import sys

for _p in ("/opt/trn_rl_repo", "/root/.axon_site/_ro/trn_rl_repo"):
    if _p not in sys.path:
        sys.path.insert(0, _p)

from concourse import bass, mybir
from concourse.bass_utils import run_bass_kernel_spmd

F16 = np.float16

N, K, NZ, H = 5000, 20, 128, 1024
NCORES = 8
VPC = N // NCORES            # 625 vertices per core
RLOC = VPC * 3               # 1875 live rows per core
NCH = 15                     # (p,a)-row chunks of 128 per core
RPAD = NCH * 128             # 1920 padded rows per core
SCALE = 2.0 / (N * K)


def build_graph(nt, na):
    """nt K-tiles over na active hidden units; last tile may be partial."""
    tk = [min(128, na - 128 * t) for t in range(nt)]
    nc = bass.Bass(target_bir_lowering=False)

    f32 = mybir.dt.float32
    f16 = mybir.dt.float16

    ut_p = nc.declare_dram_parameter("ut", [128, nt * 128], f16, isOutput=False)
    w2a = nc.declare_dram_parameter(
        "w2a", [128, NCH, 2, nt, 128], f16, isOutput=False
    )
    out_p = nc.declare_dram_parameter("out", [128, 128], f32, isOutput=True)

    from contextlib import ExitStack

    with ExitStack() as ctx:
        block = ctx.enter_context(nc.Block(no_gpsimd_drain=True))
        sem_ut = ctx.enter_context(nc.semaphore("sem_ut"))
        sem_x = ctx.enter_context(nc.semaphore("sem_x"))
        sem_xc = ctx.enter_context(nc.semaphore("sem_xc"))
        sem_h = ctx.enter_context(nc.semaphore("sem_h"))
        sem_fin = ctx.enter_context(nc.semaphore("sem_fin"))
        sem_outd = ctx.enter_context(nc.semaphore("sem_outd"))
        semw = [ctx.enter_context(nc.semaphore(f"semw{i}")) for i in range(NCH)]
        semw0x = ctx.enter_context(nc.semaphore("semw0x"))
        wthr = 16
        sb_ut = ctx.enter_context(nc.sbuf_tensor("sb_ut", [128, nt * 128], f16))
        sb_w2a = ctx.enter_context(
            nc.sbuf_tensor("sb_w2a", [128, NCH, 2, nt, 128], f16)
        )
        sb_xt = ctx.enter_context(nc.sbuf_tensor("sb_xt", [128, NCH * 128], f16))
        sb_yt = ctx.enter_context(nc.sbuf_tensor("sb_yt", [128, NCH * 128], f16))
        sb_out = ctx.enter_context(nc.sbuf_tensor("sb_out", [128, 128], f32))
        psXa = ctx.enter_context(nc.psum_tensor("psXa", [128, 128], f32))
        psXb = ctx.enter_context(nc.psum_tensor("psXb", [128, 128], f32))
        psYa = ctx.enter_context(nc.psum_tensor("psYa", [128, 128], f32))
        psYb = ctx.enter_context(nc.psum_tensor("psYb", [128, 128], f32))
        psH = ctx.enter_context(nc.psum_tensor("psH", [128, 128], f32))
        psW = ctx.enter_context(nc.psum_tensor("psW", [128, 128], f32))
        psX = [psXa, psXb]
        psY = [psYa, psYb]

        def _chunk_dma(eng, ch):
            eng.dma_start(
                out=sb_w2a[:, ch, :, :, :], in_=w2a[:, ch, :, :, :]
            ).then_inc(semw[ch], 16)

        @block.scalar
        def _(scalar: bass.BassScalarEngine):
            # U_active first on the ACT HWDGE ring, then its chunk share
            scalar.dma_start(out=sb_ut[:, :], in_=ut_p[:, :]).then_inc(sem_ut, 16)
            for ch in range(NCH):
                if ch % 3 == 1:
                    _chunk_dma(scalar, ch)


        @block.sync
        def _(sync: bass.BassEngine):
            sync.dma_start(
                out=sb_w2a[:, 0, 0, :, :], in_=w2a[:, 0, 0, :, :]
            ).then_inc(semw0x, 16)
            sync.dma_start(
                out=sb_w2a[:, 0, 1, :, :], in_=w2a[:, 0, 1, :, :]
            ).then_inc(semw[0], 16)
            for ch in range(NCH):
                if ch % 3 == 0 and ch != 0:
                    _chunk_dma(sync, ch)
            sync.wait_ge(sem_fin, 1)
            sync.dma_start(out=out_p[:, :], in_=sb_out[:, :]).then_inc(sem_outd, 16)
            sync.wait_ge(sem_outd, 16)

        @block.gpsimd
        def _(gpsimd: bass.BassGpSimd):
            for ch in range(NCH):
                if ch % 3 == 2:
                    _chunk_dma(gpsimd, ch)

        @block.tensor
        def _(tensor: bass.BassTensorEngine):
            tensor.wait_ge(sem_ut, 16)
            # HAM warmup while chunk 0 is still in flight
            for w in range(24):
                tensor.matmul(
                    psW[:, :],
                    lhsT=sb_ut[:, 0:128],
                    rhs=sb_ut[:, 0:128],
                    start=True,
                    stop=True,
                )
            def _s3(ch):
                return tensor.matmul(
                    psH[:, :],
                    lhsT=sb_xt[:, ch * 128 : (ch + 1) * 128],
                    rhs=sb_yt[:, ch * 128 : (ch + 1) * 128],
                    start=(ch == 0),
                    stop=(ch == NCH - 1),
                )

            for ch in range(NCH):
                if ch >= 2:
                    tensor.wait_ge(sem_xc, 2 * (ch - 1))
                    _s3(ch - 2)  # stage-3 for ch-2 (its copies are done)
                tensor.wait_ge(semw0x if ch == 0 else semw[ch], wthr)
                for t in range(nt):
                    ins = tensor.matmul(
                        psX[ch % 2][:, :],
                        lhsT=sb_w2a[: tk[t], ch, 0, t, :],
                        rhs=sb_ut[: tk[t], t * 128 : (t + 1) * 128],
                        start=(t == 0),
                        stop=(t == nt - 1),
                    )
                ins.then_inc(sem_x, 1)
                if ch == 0:
                    tensor.wait_ge(semw[0], wthr)
                for t in range(nt):
                    ins = tensor.matmul(
                        psY[ch % 2][:, :],
                        lhsT=sb_w2a[: tk[t], ch, 1, t, :],
                        rhs=sb_ut[: tk[t], t * 128 : (t + 1) * 128],
                        start=(t == 0),
                        stop=(t == nt - 1),
                    )
                ins.then_inc(sem_x, 1)
            for ch in (NCH - 2, NCH - 1):
                tensor.wait_ge(sem_xc, 2 * (ch + 1))
                ins = _s3(ch)
            ins.then_inc(sem_h, 1)

        @block.vector
        def _(vector: bass.BassVectorEngine):
            # PSUM -> SBUF f16 copies of stage-1 chunks (X then Y per chunk)
            for ch in range(NCH):
                vector.wait_ge(sem_x, 2 * ch + 1)
                vector.tensor_copy(
                    sb_xt[:, ch * 128 : (ch + 1) * 128], psX[ch % 2][:, :]
                ).then_inc(sem_xc, 1)
                vector.wait_ge(sem_x, 2 * ch + 2)
                vector.tensor_copy(
                    sb_yt[:, ch * 128 : (ch + 1) * 128], psY[ch % 2][:, :]
                ).then_inc(sem_xc, 1)
            vector.wait_ge(sem_h, 1)
            vector.tensor_copy(sb_out[:, :], psH[:, :]).then_inc(sem_fin, 1)

    return nc


def prep_inputs(code, xyz1, weightMatrix, W1, b1, W2, b2, neighborsMatrix, numNeighbors):
    """Host-side sharding/layout prep. Returns (in_maps, nt)."""
    code = np.asarray(code, np.float64)
    W1 = np.asarray(W1, np.float64)
    W2 = np.asarray(W2, np.float32)
    b1 = np.asarray(b1, np.float64)
    wM = np.asarray(weightMatrix, np.float32)
    nbr = np.asarray(neighborsMatrix, np.int64)
    nn = np.asarray(numNeighbors, np.int64)

    mask = (np.arange(K)[None, :] < nn[:, None]).astype(np.float64)
    wt = np.asarray(wM, np.float64) * mask              # [N, K]

    # relu mask -> active hidden units (zero columns of U drop out exactly)
    z = (code @ W1 + b1)[0]
    act = np.where(z > 0)[0]
    na = len(act)
    nt = max(1, (na + 127) // 128)
    HP = nt * 128

    # W2L = W2 (L (x) I3)
    W2vT = np.ascontiguousarray(
        W2.astype(np.float32).reshape(H, N, 3).transpose(1, 2, 0)
    )                                                   # [N, 3, H]
    deg_out = wt.sum(1)
    deg_in = np.zeros(N)
    np.add.at(deg_in, nbr.ravel(), wt.ravel())
    d_tot = (deg_out + deg_in).astype(np.float32)

    W2LvT = W2vT * d_tot[:, None, None]
    wt32 = wt.astype(np.float32)
    for j in range(K):
        nj, wj = nbr[:, j], wt32[:, j]
        W2LvT -= wj[:, None, None] * W2vT[nj]                    # S term
        np.add.at(W2LvT, nj, -(wj[:, None, None] * W2vT))        # S^T term

    # active-row selection, padded to HP
    W2a = np.zeros((HP, N * 3), np.float32)
    W2a[:na] = W2.reshape(H, N * 3)[act]
    W2La = np.zeros((HP, N * 3), np.float32)
    W2La[:na] = W2LvT.transpose(2, 0, 1).reshape(H, N * 3)[act]

    # U_active^T tiles: ut[p, t*128+k] = W1[k, act[t*128+p]]  (pad rows zero)
    ut_h = np.zeros((HP, NZ), np.float32)
    ut_h[:na] = W1.T[act]
    ut_h = np.ascontiguousarray(
        ut_h.reshape(nt, 128, NZ).transpose(1, 0, 2).reshape(128, nt * NZ)
    ).astype(F16)

    def col_block(M, c):
        blk = np.zeros((HP, RPAD), np.float32)
        blk[:, :RLOC] = M[:, 3 * c * VPC : 3 * c * VPC + RLOC]
        # [part, ch, t, col] = blk[t*128+part, ch*128+col]
        return blk.reshape(nt, 128, NCH, 128).transpose(1, 2, 0, 3)

    in_maps = []
    for c in range(NCORES):
        both = np.stack([col_block(W2a, c), col_block(W2La, c)], axis=2)
        in_maps.append(
            {
                "ut": ut_h,
                "w2a": np.ascontiguousarray(both).astype(F16),
            }
        )
    return in_maps, nt, na


_CACHED = {}


def run_on_hw(in_maps, nt, na, trace=False):
    if (nt, na) not in _CACHED:
        _CACHED[(nt, na)] = build_graph(nt, na)
    res = run_bass_kernel_spmd(
        _CACHED[(nt, na)], in_maps, core_ids=list(range(NCORES)), trace=trace
    )
    return res


def assemble(parts):
    m = np.sum([np.asarray(p, np.float64) for p in parts], axis=0)
    return (m * SCALE).astype(np.float32)


def kernel(**inputs):
    in_maps, nt, na = prep_inputs(**inputs)
    res = run_on_hw(in_maps, nt, na)
    return assemble([res.results[c]["out"] for c in range(NCORES)])


if __name__ == "__main__":
    import reference

    inputs = {k: np.asarray(v) for k, v in reference.setup_inputs().items()}
    out = kernel(**inputs)
    print("out shape", out.shape, "absmax", np.abs(out).max())


# revision 40
# speedup vs baseline: 1.2249x; 1.1112x over previous
"""Trainium2 Bass kernel: analytical Hessian of the ARAP energy w.r.t. a latent code.

Math (derived from the reference, exact because relu'' == 0 a.e.):
    wt[p,j] = weightMatrix[p,j] * (j < numNeighbors[p])          [N, K]
    s       = (code @ W1 + b1 > 0)                               [H]
    X       = (W1 * s) @ W2   viewed [NZ, N*3]                   (the Jacobian d recon/d code)
    L       = D - S - S^T     (graph Laplacian; S[p, n[p,j]] += wt[p,j],
                               D = diag(rowsum(S) + colsum(S)))
    Hess    = (2/(N*K)) * X (L (x) I3) X^T                       [NZ, NZ]

Two structural identities shape the kernel:
  1. X (L (x) I3) = U @ (W2 (L (x) I3)): the sparse Laplacian application is a
     fixed recombination of W2's columns by the static, input-derived edge
     weights -- precomputed once on the host as W2L (the device's hardware
     gather paths are unusable in this stack; the matmul mass stays on device).
  2. U = W1 * s has zero columns wherever the relu is inactive -- those rows of
     W2 / W2L contribute nothing, so only the ~H/2 active rows are shipped and
     multiplied (structured sparsity; prep_inputs derives the mask from the
     actual runtime inputs, so this is exact for any inputs).

Per core (vertices column-sharded, 625/core; HP = padded active-row count):
    stage 1a:  XT_c = (W2 active block)^T  @ U_active    NCH chunks x HP/128 K-tiles
    stage 1b:  YT_c = (W2L active block)^T @ U_active    NCH chunks x HP/128 K-tiles
    stage 3 :  psH += XT_c (contract rows) YT_c          NCH accumulating matmuls
Per-core partial Hessians are summed on the host (times 2/(N*K)).
W2/W2L chunks stream via per-chunk DMAs so TensorE starts ~2us in.
"""

import numpy as np

import sys

for _p in ("/opt/trn_rl_repo", "/root/.axon_site/_ro/trn_rl_repo"):
    if _p not in sys.path:
        sys.path.insert(0, _p)

from concourse import bass, mybir
from concourse.bass_utils import run_bass_kernel_spmd

F16 = np.float16

N, K, NZ, H = 5000, 20, 128, 1024
NCORES = 8
VPC = N // NCORES            # 625 vertices per core
RLOC = VPC * 3               # 1875 live rows per core
NCH = 15                     # (p,a)-row chunks of 128 per core
RPAD = NCH * 128             # 1920 padded rows per core
SCALE = 2.0 / (N * K)


def build_graph(nt, na):
    """nt K-tiles over na active hidden units; last tile may be partial."""
    tk = [min(128, na - 128 * t) for t in range(nt)]
    nc = bass.Bass(target_bir_lowering=False)

    f32 = mybir.dt.float32
    f16 = mybir.dt.float16

    ut_p = nc.declare_dram_parameter("ut", [128, nt * 128], f16, isOutput=False)
    w2a = nc.declare_dram_parameter(
        "w2a", [128, NCH, 2, nt, 128], f16, isOutput=False
    )
    out_p = nc.declare_dram_parameter("out", [128, 128], f32, isOutput=True)

    from contextlib import ExitStack

    with ExitStack() as ctx:
        block = ctx.enter_context(nc.Block(no_gpsimd_drain=True))
        sem_ut = ctx.enter_context(nc.semaphore("sem_ut"))
        sem_x = ctx.enter_context(nc.semaphore("sem_x"))
        sem_xc = ctx.enter_context(nc.semaphore("sem_xc"))
        sem_h = ctx.enter_context(nc.semaphore("sem_h"))
        sem_fin = ctx.enter_context(nc.semaphore("sem_fin"))
        sem_outd = ctx.enter_context(nc.semaphore("sem_outd"))
        semw = [ctx.enter_context(nc.semaphore(f"semw{i}")) for i in range(NCH)]
        semw0x = ctx.enter_context(nc.semaphore("semw0x"))
        wthr = 16
        sb_ut = ctx.enter_context(nc.sbuf_tensor("sb_ut", [128, nt * 128], f16))
        sb_w2a = ctx.enter_context(
            nc.sbuf_tensor("sb_w2a", [128, NCH, 2, nt, 128], f16)
        )
        sb_xt = ctx.enter_context(nc.sbuf_tensor("sb_xt", [128, NCH * 128], f16))
        sb_yt = ctx.enter_context(nc.sbuf_tensor("sb_yt", [128, NCH * 128], f16))
        sb_out = ctx.enter_context(nc.sbuf_tensor("sb_out", [128, 128], f32))
        psXa = ctx.enter_context(nc.psum_tensor("psXa", [128, 128], f32))
        psXb = ctx.enter_context(nc.psum_tensor("psXb", [128, 128], f32))
        psYa = ctx.enter_context(nc.psum_tensor("psYa", [128, 128], f32))
        psYb = ctx.enter_context(nc.psum_tensor("psYb", [128, 128], f32))
        psH = ctx.enter_context(nc.psum_tensor("psH", [128, 128], f32))
        psW = ctx.enter_context(nc.psum_tensor("psW", [128, 128], f32))
        psX = [psXa, psXb]
        psY = [psYa, psYb]

        def _chunk_dma(eng, ch):
            eng.dma_start(
                out=sb_w2a[:, ch, :, :, :], in_=w2a[:, ch, :, :, :]
            ).then_inc(semw[ch], 16)

        @block.scalar
        def _(scalar: bass.BassScalarEngine):
            # U_active first on the ACT HWDGE ring, then its chunk share
            scalar.dma_start(out=sb_ut[:, :], in_=ut_p[:, :]).then_inc(sem_ut, 16)
            for ch in range(NCH):
                if ch % 3 == 1:
                    _chunk_dma(scalar, ch)


        @block.sync
        def _(sync: bass.BassEngine):
            sync.dma_start(
                out=sb_w2a[:, 0, 0, :, :], in_=w2a[:, 0, 0, :, :]
            ).then_inc(semw0x, 16)
            sync.dma_start(
                out=sb_w2a[:, 0, 1, :, :], in_=w2a[:, 0, 1, :, :]
            ).then_inc(semw[0], 16)
            for ch in range(NCH):
                if ch % 3 == 0 and ch != 0:
                    _chunk_dma(sync, ch)
            sync.wait_ge(sem_fin, 1)
            sync.dma_start(out=out_p[:, :], in_=sb_out[:, :]).then_inc(sem_outd, 16)
            sync.wait_ge(sem_outd, 16)

        @block.gpsimd
        def _(gpsimd: bass.BassGpSimd):
            for ch in range(NCH):
                if ch % 3 == 2:
                    _chunk_dma(gpsimd, ch)

        @block.tensor
        def _(tensor: bass.BassTensorEngine):
            tensor.wait_ge(sem_ut, 16)
            # HAM warmup while chunk 0 is still in flight
            for w in range(24):
                tensor.matmul(
                    psW[:, :],
                    lhsT=sb_ut[:, 0:128],
                    rhs=sb_ut[:, 0:128],
                    start=True,
                    stop=True,
                )
            def _s3(ch):
                return tensor.matmul(
                    psH[:, :],
                    lhsT=sb_xt[:, ch * 128 : (ch + 1) * 128],
                    rhs=sb_yt[:, ch * 128 : (ch + 1) * 128],
                    start=(ch == 0),
                    stop=(ch == NCH - 1),
                )

            for ch in range(NCH):
                if ch >= 2:
                    tensor.wait_ge(sem_xc, 2 * (ch - 1))
                tensor.wait_ge(semw0x if ch == 0 else semw[ch], wthr)
                for t in range(nt):
                    ins = tensor.matmul(
                        psX[ch % 2][:, :],
                        lhsT=sb_w2a[:, ch, 0, t, :],
                        rhs=sb_ut[:, t * 128 : (t + 1) * 128],
                        start=(t == 0),
                        stop=(t == nt - 1),
                    )
                ins.then_inc(sem_x, 1)
                if ch == 0:
                    tensor.wait_ge(semw[0], wthr)
                for t in range(nt):
                    ins = tensor.matmul(
                        psY[ch % 2][:, :],
                        lhsT=sb_w2a[:, ch, 1, t, :],
                        rhs=sb_ut[:, t * 128 : (t + 1) * 128],
                        start=(t == 0),
                        stop=(t == nt - 1),
                    )
                ins.then_inc(sem_x, 1)
            for ch in range(NCH):
                tensor.wait_ge(sem_xc, 2 * (ch + 1))
                ins = _s3(ch)
            ins.then_inc(sem_h, 1)

        @block.vector
        def _(vector: bass.BassVectorEngine):
            # PSUM -> SBUF f16 copies of stage-1 chunks (X then Y per chunk)
            for ch in range(NCH):
                vector.wait_ge(sem_x, 2 * ch + 1)
                vector.tensor_copy(
                    sb_xt[:, ch * 128 : (ch + 1) * 128], psX[ch % 2][:, :]
                ).then_inc(sem_xc, 1)
                vector.wait_ge(sem_x, 2 * ch + 2)
                vector.tensor_copy(
                    sb_yt[:, ch * 128 : (ch + 1) * 128], psY[ch % 2][:, :]
                ).then_inc(sem_xc, 1)
            vector.wait_ge(sem_h, 1)
            vector.tensor_copy(sb_out[:, :], psH[:, :]).then_inc(sem_fin, 1)

    return nc


def prep_inputs(code, xyz1, weightMatrix, W1, b1, W2, b2, neighborsMatrix, numNeighbors):
    """Host-side sharding/layout prep. Returns (in_maps, nt)."""
    code = np.asarray(code, np.float64)
    W1 = np.asarray(W1, np.float64)
    W2 = np.asarray(W2, np.float32)
    b1 = np.asarray(b1, np.float64)
    wM = np.asarray(weightMatrix, np.float32)
    nbr = np.asarray(neighborsMatrix, np.int64)
    nn = np.asarray(numNeighbors, np.int64)

    mask = (np.arange(K)[None, :] < nn[:, None]).astype(np.float64)
    wt = np.asarray(wM, np.float64) * mask              # [N, K]

    # relu mask -> active hidden units (zero columns of U drop out exactly)
    z = (code @ W1 + b1)[0]
    act = np.where(z > 0)[0]
    na = len(act)
    nt = max(1, (na + 127) // 128)
    HP = nt * 128

    # W2L = W2 (L (x) I3)
    W2vT = np.ascontiguousarray(
        W2.astype(np.float32).reshape(H, N, 3).transpose(1, 2, 0)
    )                                                   # [N, 3, H]
    deg_out = wt.sum(1)
    deg_in = np.zeros(N)
    np.add.at(deg_in, nbr.ravel(), wt.ravel())
    d_tot = (deg_out + deg_in).astype(np.float32)

    W2LvT = W2vT * d_tot[:, None, None]
    wt32 = wt.astype(np.float32)
    for j in range(K):
        nj, wj = nbr[:, j], wt32[:, j]
        W2LvT -= wj[:, None, None] * W2vT[nj]                    # S term
        np.add.at(W2LvT, nj, -(wj[:, None, None] * W2vT))        # S^T term

    # active-row selection, padded to HP
    W2a = np.zeros((HP, N * 3), np.float32)
    W2a[:na] = W2.reshape(H, N * 3)[act]
    W2La = np.zeros((HP, N * 3), np.float32)
    W2La[:na] = W2LvT.transpose(2, 0, 1).reshape(H, N * 3)[act]

    # U_active^T tiles: ut[p, t*128+k] = W1[k, act[t*128+p]]  (pad rows zero)
    ut_h = np.zeros((HP, NZ), np.float32)
    ut_h[:na] = W1.T[act]
    ut_h = np.ascontiguousarray(
        ut_h.reshape(nt, 128, NZ).transpose(1, 0, 2).reshape(128, nt * NZ)
    ).astype(F16)

    def col_block(M, c):
        blk = np.zeros((HP, RPAD), np.float32)
        blk[:, :RLOC] = M[:, 3 * c * VPC : 3 * c * VPC + RLOC]
        # [part, ch, t, col] = blk[t*128+part, ch*128+col]
        return blk.reshape(nt, 128, NCH, 128).transpose(1, 2, 0, 3)

    in_maps = []
    for c in range(NCORES):
        both = np.stack([col_block(W2a, c), col_block(W2La, c)], axis=2)
        in_maps.append(
            {
                "ut": ut_h,
                "w2a": np.ascontiguousarray(both).astype(F16),
            }
        )
    return in_maps, nt, na


_CACHED = {}


def run_on_hw(in_maps, nt, na, trace=False):
    if (nt, na) not in _CACHED:
        _CACHED[(nt, na)] = build_graph(nt, na)
    res = run_bass_kernel_spmd(
        _CACHED[(nt, na)], in_maps, core_ids=list(range(NCORES)), trace=trace
    )
    return res


def assemble(parts):
    m = np.sum([np.asarray(p, np.float64) for p in parts], axis=0)
    return (m * SCALE).astype(np.float32)


def kernel(**inputs):
    in_maps, nt, na = prep_inputs(**inputs)
    res = run_on_hw(in_maps, nt, na)
    return assemble([res.results[c]["out"] for c in range(NCORES)])


if __name__ == "__main__":
    import reference

    inputs = {k: np.asarray(v) for k, v in reference.setup_inputs().items()}
    out = kernel(**inputs)
    print("out shape", out.shape, "absmax", np.abs(out).max())


# revision 42
# speedup vs baseline: 1.4182x; 1.1578x over previous
"""Trainium2 Bass kernel: analytical Hessian of the ARAP energy w.r.t. a latent code.

Math (derived from the reference, exact because relu'' == 0 a.e.):
    wt[p,j] = weightMatrix[p,j] * (j < numNeighbors[p])          [N, K]
    s       = (code @ W1 + b1 > 0)                               [H]
    X       = (W1 * s) @ W2   viewed [NZ, N*3]                   (the Jacobian d recon/d code)
    L       = D - S - S^T     (graph Laplacian; S[p, n[p,j]] += wt[p,j],
                               D = diag(rowsum(S) + colsum(S)))
    Hess    = (2/(N*K)) * X (L (x) I3) X^T                       [NZ, NZ]

Two structural identities shape the kernel:
  1. X (L (x) I3) = U @ (W2 (L (x) I3)): the sparse Laplacian application is a
     fixed recombination of W2's columns by the static, input-derived edge
     weights -- precomputed once on the host as W2L (the device's hardware
     gather paths are unusable in this stack; the matmul mass stays on device).
  2. U = W1 * s has zero columns wherever the relu is inactive -- those rows of
     W2 / W2L contribute nothing, so only the ~H/2 active rows are shipped and
     multiplied (structured sparsity; prep_inputs derives the mask from the
     actual runtime inputs, so this is exact for any inputs).

Per core (vertices column-sharded, 625/core; HP = padded active-row count):
    stage 1a:  XT_c = (W2 active block)^T  @ U_active    NCH chunks x HP/128 K-tiles
    stage 1b:  YT_c = (W2L active block)^T @ U_active    NCH chunks x HP/128 K-tiles
    stage 3 :  psH += XT_c (contract rows) YT_c          NCH accumulating matmuls
Per-core partial Hessians are summed on the host (times 2/(N*K)).
W2/W2L chunks stream via per-chunk DMAs so TensorE starts ~2us in.
"""

import numpy as np

import sys

for _p in ("/opt/trn_rl_repo", "/root/.axon_site/_ro/trn_rl_repo"):
    if _p not in sys.path:
        sys.path.insert(0, _p)

from concourse import bass, mybir
from concourse.bass_utils import run_bass_kernel_spmd

F16 = np.float16

N, K, NZ, H = 5000, 20, 128, 1024
NCORES = 8
VPC = N // NCORES            # 625 vertices per core
RLOC = VPC * 3               # 1875 live rows per core
NCH = 15                     # (p,a)-row chunks of 128 per core
RPAD = NCH * 128             # 1920 padded rows per core
SCALE = 2.0 / (N * K)


def build_graph(nt, na):
    """nt K-tiles over na active hidden units; last tile may be partial."""
    tk = [min(128, na - 128 * t) for t in range(nt)]
    nc = bass.Bass(target_bir_lowering=False)

    f32 = mybir.dt.float32
    f16 = mybir.dt.float16

    ut_p = nc.declare_dram_parameter("ut", [128, nt * 128], f16, isOutput=False)
    w2a = nc.declare_dram_parameter(
        "w2a", [128, NCH, 2, nt, 128], f16, isOutput=False
    )
    out_p = nc.declare_dram_parameter("out", [128, 128], f32, isOutput=True)

    from contextlib import ExitStack

    with ExitStack() as ctx:
        block = ctx.enter_context(nc.Block(no_gpsimd_drain=True))
        sem_ut = ctx.enter_context(nc.semaphore("sem_ut"))
        sem_x = ctx.enter_context(nc.semaphore("sem_x"))
        sem_xc = ctx.enter_context(nc.semaphore("sem_xc"))
        sem_h = ctx.enter_context(nc.semaphore("sem_h"))
        sem_fin = ctx.enter_context(nc.semaphore("sem_fin"))
        sem_outd = ctx.enter_context(nc.semaphore("sem_outd"))
        semw = [ctx.enter_context(nc.semaphore(f"semw{i}")) for i in range(NCH)]
        semwx = [ctx.enter_context(nc.semaphore(f"semwx{i}")) for i in range(NCH)]
        wthr = 16
        sb_ut = ctx.enter_context(nc.sbuf_tensor("sb_ut", [128, nt * 128], f16))
        sb_w2a = ctx.enter_context(
            nc.sbuf_tensor("sb_w2a", [128, NCH, 2, nt, 128], f16)
        )
        sb_xt = ctx.enter_context(nc.sbuf_tensor("sb_xt", [128, NCH * 128], f16))
        sb_yt = ctx.enter_context(nc.sbuf_tensor("sb_yt", [128, NCH * 128], f16))
        sb_out = ctx.enter_context(nc.sbuf_tensor("sb_out", [128, 128], f32))
        psXa = ctx.enter_context(nc.psum_tensor("psXa", [128, 128], f32))
        psXb = ctx.enter_context(nc.psum_tensor("psXb", [128, 128], f32))
        psYa = ctx.enter_context(nc.psum_tensor("psYa", [128, 128], f32))
        psYb = ctx.enter_context(nc.psum_tensor("psYb", [128, 128], f32))
        psH = ctx.enter_context(nc.psum_tensor("psH", [128, 128], f32))
        psW = ctx.enter_context(nc.psum_tensor("psW", [128, 128], f32))
        psX = [psXa, psXb]
        psY = [psYa, psYb]

        def _chunk_dma(eng, ch):
            eng.dma_start(
                out=sb_w2a[:, ch, 0, :, :], in_=w2a[:, ch, 0, :, :]
            ).then_inc(semwx[ch], 16)
            eng.dma_start(
                out=sb_w2a[:, ch, 1, :, :], in_=w2a[:, ch, 1, :, :]
            ).then_inc(semw[ch], 16)

        @block.scalar
        def _(scalar: bass.BassScalarEngine):
            # U_active first on the ACT HWDGE ring, then its chunk share
            scalar.dma_start(out=sb_ut[:, :], in_=ut_p[:, :]).then_inc(sem_ut, 16)
            for ch in range(NCH):
                if ch % 3 == 1:
                    _chunk_dma(scalar, ch)


        @block.sync
        def _(sync: bass.BassEngine):
            for ch in range(NCH):
                if ch % 3 == 0:
                    _chunk_dma(sync, ch)
            sync.wait_ge(sem_fin, 1)
            sync.dma_start(out=out_p[:, :], in_=sb_out[:, :]).then_inc(sem_outd, 16)
            sync.wait_ge(sem_outd, 16)

        @block.gpsimd
        def _(gpsimd: bass.BassGpSimd):
            for ch in range(NCH):
                if ch % 3 == 2:
                    _chunk_dma(gpsimd, ch)

        @block.tensor
        def _(tensor: bass.BassTensorEngine):
            tensor.wait_ge(sem_ut, 16)
            # HAM warmup while chunk 0 is still in flight
            for w in range(24):
                tensor.matmul(
                    psW[:, :],
                    lhsT=sb_ut[:, 0:128],
                    rhs=sb_ut[:, 0:128],
                    start=True,
                    stop=True,
                )
            def _s3(ch):
                return tensor.matmul(
                    psH[:, :],
                    lhsT=sb_xt[:, ch * 128 : (ch + 1) * 128],
                    rhs=sb_yt[:, ch * 128 : (ch + 1) * 128],
                    start=(ch == 0),
                    stop=(ch == NCH - 1),
                )

            for ch in range(NCH):
                if ch >= 2:
                    tensor.wait_ge(sem_xc, 2 * (ch - 1))
                tensor.wait_ge(semwx[ch], 16)
                for t in range(nt):
                    ins = tensor.matmul(
                        psX[ch % 2][:, :],
                        lhsT=sb_w2a[:, ch, 0, t, :],
                        rhs=sb_ut[:, t * 128 : (t + 1) * 128],
                        start=(t == 0),
                        stop=(t == nt - 1),
                    )
                ins.then_inc(sem_x, 1)
                tensor.wait_ge(semw[ch], 16)
                for t in range(nt):
                    ins = tensor.matmul(
                        psY[ch % 2][:, :],
                        lhsT=sb_w2a[:, ch, 1, t, :],
                        rhs=sb_ut[:, t * 128 : (t + 1) * 128],
                        start=(t == 0),
                        stop=(t == nt - 1),
                    )
                ins.then_inc(sem_x, 1)
            for ch in range(NCH):
                tensor.wait_ge(sem_xc, 2 * (ch + 1))
                ins = _s3(ch)
            ins.then_inc(sem_h, 1)

        @block.vector
        def _(vector: bass.BassVectorEngine):
            # PSUM -> SBUF f16 copies of stage-1 chunks (X then Y per chunk)
            for ch in range(NCH):
                vector.wait_ge(sem_x, 2 * ch + 1)
                vector.tensor_copy(
                    sb_xt[:, ch * 128 : (ch + 1) * 128], psX[ch % 2][:, :]
                ).then_inc(sem_xc, 1)
                vector.wait_ge(sem_x, 2 * ch + 2)
                vector.tensor_copy(
                    sb_yt[:, ch * 128 : (ch + 1) * 128], psY[ch % 2][:, :]
                ).then_inc(sem_xc, 1)
            vector.wait_ge(sem_h, 1)
            vector.tensor_copy(sb_out[:, :], psH[:, :]).then_inc(sem_fin, 1)

    return nc


def prep_inputs(code, xyz1, weightMatrix, W1, b1, W2, b2, neighborsMatrix, numNeighbors):
    """Host-side sharding/layout prep. Returns (in_maps, nt)."""
    code = np.asarray(code, np.float64)
    W1 = np.asarray(W1, np.float64)
    W2 = np.asarray(W2, np.float32)
    b1 = np.asarray(b1, np.float64)
    wM = np.asarray(weightMatrix, np.float32)
    nbr = np.asarray(neighborsMatrix, np.int64)
    nn = np.asarray(numNeighbors, np.int64)

    mask = (np.arange(K)[None, :] < nn[:, None]).astype(np.float64)
    wt = np.asarray(wM, np.float64) * mask              # [N, K]

    # relu mask -> active hidden units (zero columns of U drop out exactly)
    z = (code @ W1 + b1)[0]
    act = np.where(z > 0)[0]
    na = len(act)
    nt = max(1, (na + 127) // 128)
    HP = nt * 128

    # W2L = W2 (L (x) I3)
    W2vT = np.ascontiguousarray(
        W2.astype(np.float32).reshape(H, N, 3).transpose(1, 2, 0)
    )                                                   # [N, 3, H]
    deg_out = wt.sum(1)
    deg_in = np.zeros(N)
    np.add.at(deg_in, nbr.ravel(), wt.ravel())
    d_tot = (deg_out + deg_in).astype(np.float32)

    W2LvT = W2vT * d_tot[:, None, None]
    wt32 = wt.astype(np.float32)
    for j in range(K):
        nj, wj = nbr[:, j], wt32[:, j]
        W2LvT -= wj[:, None, None] * W2vT[nj]                    # S term
        np.add.at(W2LvT, nj, -(wj[:, None, None] * W2vT))        # S^T term

    # active-row selection, padded to HP
    W2a = np.zeros((HP, N * 3), np.float32)
    W2a[:na] = W2.reshape(H, N * 3)[act]
    W2La = np.zeros((HP, N * 3), np.float32)
    W2La[:na] = W2LvT.transpose(2, 0, 1).reshape(H, N * 3)[act]

    # U_active^T tiles: ut[p, t*128+k] = W1[k, act[t*128+p]]  (pad rows zero)
    ut_h = np.zeros((HP, NZ), np.float32)
    ut_h[:na] = W1.T[act]
    ut_h = np.ascontiguousarray(
        ut_h.reshape(nt, 128, NZ).transpose(1, 0, 2).reshape(128, nt * NZ)
    ).astype(F16)

    def col_block(M, c):
        blk = np.zeros((HP, RPAD), np.float32)
        blk[:, :RLOC] = M[:, 3 * c * VPC : 3 * c * VPC + RLOC]
        # [part, ch, t, col] = blk[t*128+part, ch*128+col]
        return blk.reshape(nt, 128, NCH, 128).transpose(1, 2, 0, 3)

    in_maps = []
    for c in range(NCORES):
        both = np.stack([col_block(W2a, c), col_block(W2La, c)], axis=2)
        in_maps.append(
            {
                "ut": ut_h,
                "w2a": np.ascontiguousarray(both).astype(F16),
            }
        )
    return in_maps, nt, na


_CACHED = {}


def run_on_hw(in_maps, nt, na, trace=False):
    if (nt, na) not in _CACHED:
        _CACHED[(nt, na)] = build_graph(nt, na)
    res = run_bass_kernel_spmd(
        _CACHED[(nt, na)], in_maps, core_ids=list(range(NCORES)), trace=trace
    )
    return res


def assemble(parts):
    m = np.sum([np.asarray(p, np.float64) for p in parts], axis=0)
    return (m * SCALE).astype(np.float32)


def kernel(**inputs):
    in_maps, nt, na = prep_inputs(**inputs)
    res = run_on_hw(in_maps, nt, na)
    return assemble([res.results[c]["out"] for c in range(NCORES)])


if __name__ == "__main__":
    import reference

    inputs = {k: np.asarray(v) for k, v in reference.setup_inputs().items()}
    out = kernel(**inputs)
    print("out shape", out.shape, "absmax", np.abs(out).max())
